# revision 33
# baseline (speedup 1.0000x reference)
"""Chamfer loss (nn_ChamferLoss) Bass kernel for Trainium2.

Data-parallel over the batch dim: 8 batches, one NeuronCore each. Per core
(one batch, clouds A = X[b].T and B = Y[b].T, each 4096 x 3 fp32):

  The NEGATED 4096x4096 squared-distance matrix u = -t is produced
  tile-by-tile in PSUM by one matmul per tile whose contraction rows
  encode the whole formula (see _operands): bf16 hi/lo decomposition
  (K=30) reproduces fp32 numerics at full bf16 matmul speed. Negation is
  folded into the operands so that both reductions become MAX.

  Single-pass reduction (mode "sp"): each [128, 4096] row-block of u is
  visited ONCE (the original kernel built the matrix twice and
  tensor_reduce'd all 2x16.7M elements on DVE at 1 elem/cycle -> ~262 us):

    * ScalarE evacuates every PSUM tile to SBUF, converting fp32 -> bf16
      (it is the only engine besides DVE that can read PSUM; GpSimd
      elementwise ops do not codegen for TRN2's Pool engine at all).
    * Row stream, d2[n] = min_m: a CUSTOM DVE uop PAIR_MAX_REDUCE_ANT
      (registered at import into the ant-dve table: body = max(Src0,Src1),
      accum = MAX) folds a PAIR of bf16 tiles AND max-reduces the result
      into [P, 1] in one 1x pass -- 0.5 cycles/element where stock
      tensor_reduce needs 1.0 (the stock fused-TTR min path faults on TRN2
      hardware, and TENSOR_TENSOR_SCAN, though HW-verified, measures ~2x
      slower than tensor_reduce).
    * Column stream, d1[m] = min_n: elementwise bf16 running max over the
      32 row-blocks into a [128, 4096] accumulator via stock
      tensor_tensor at 2x bf16 rate (block 0 is a 4x tensor_copy,
      doubling as the init).
    * Tail: PE transposes the accumulator through an identity matmul and
      DVE max-reduces the 128 partials per column; only [P, 2] per-lane
      sums leave the device (an earlier variant DMA'd the 1 MB accumulator
      out per rep and throttled to ~6 GB/s on the output queue).

  The host (numpy, float64) sums the per-lane partials and negates.

Measured (8 cores, axon TRN2): loss rel err 9.9e-6 vs the jax reference,
~170 us per kernel execution (in-NEFF repeat-loop marginal cost; the
original two-pass kernel measures ~262 us under the same protocol).
"""

import numpy as np

B, C, N = 8, 3, 4096
P = 128      # partition width / rows per block
NTILE = 4    # PSUM tiles per row-block (each W = n/NTILE wide)

_cache = {}


_PMR = None


def _register_pmr():
    """Register the PAIR_MAX_REDUCE_ANT custom DVE op (process-local):

        out[p, k]    = max(in0[p, k], in1[p, k])
        accum_out[p] = max_k out[p, k]        (seed = -FLT_MAX)

    One 1x DVE pass folds two SBUF tiles into a per-partition max -- the
    fused row-reduction whose stock TTR min-ucode faults on TRN2. Run on
    the negated distance matrix, accum_out = -(row min over both tiles).
    """
    global _PMR
    if _PMR is not None:
        return _PMR
    import concourse.dve_ops as dvo
    from concourse.dve_spec import AluOp, Spec, Src0, Src1, lower, maxx, _has_src1
    from concourse.dve_uop import DveOpSpec

    name = "PAIR_MAX_REDUCE_ANT"
    if name in dvo._SUB_OPCODE_FOR_NAME:
        _PMR = next(op for op in dvo.OPS if op.name == name)
        return _PMR

    def _ref(in0, in1, s0, s1, imm2):
        b = np.maximum(in0.astype(np.float32), in1.astype(np.float32))
        return b, b.reshape(b.shape[0], -1).max(axis=-1, keepdims=True)

    def mkspec():
        return Spec(body=maxx(Src0, Src1), accum=AluOp.MAX, reference=_ref)

    row = dvo._CUSTOM_DVE_ROW_BASE + len(dvo.OPS)
    assert row < 0x20
    dvo._SUB_OPCODE_FOR_NAME[name] = row
    shas = {}
    for ver in ("v3", "v4"):
        spec = mkspec()
        s = DveOpSpec(
            name=name, opcode=row, uops=lower(spec, ver=ver),
            rd1_en=_has_src1(spec),
        )
        shas[ver] = s.sha(ver)
    spec = mkspec()
    _PMR = dvo.DveOp(name, spec, subdim=False, uops_sha=shas)
    dvo.OPS.append(_PMR)
    dvo.CUSTOM_DVE_SPECS[name] = spec
    return _PMR


def _operands(nc, tc, small, big, n, bf16x=True, neg=False):
    """Build lhsT/rhs matmul operands encoding the distance formula.

    Returns (lhsT1, rhs1, lhsT2, rhs2) where pass k's t = lhsTk.T @ rhsk.
    Only pass 1 is used by the single-pass kernel.
    """
    import concourse.mybir as mybir

    f32 = mybir.dt.float32
    bf16 = mybir.dt.bfloat16
    AL = mybir.AluOpType
    PF = (C * n) // P
    K = 30

    X_d, Y_d = nc._X_d, nc._Y_d
    lhsT1 = big.tile([K, n], bf16, tag="lhsT1")
    rhs1 = big.tile([K, n], bf16, tag="rhs1")

    flatX = small.tile([PF, P], f32, tag="flatX")
    flatY = small.tile([PF, P], f32, tag="flatY")
    xf_src = X_d[:].rearrange("c n -> (c n)").rearrange("(p f) -> p f", f=P)
    yf_src = Y_d[:].rearrange("c n -> (c n)").rearrange("(p f) -> p f", f=P)
    nc.sync.dma_start(out=flatX[:], in_=xf_src)
    nc.sync.dma_start(out=flatY[:], in_=yf_src)

    def ft(name, dtype):
        return small.tile([PF, P], dtype, tag=name, name=name)

    def rows(dst, g, src):
        nc.sync.dma_start(out=dst[3 * g : 3 * g + 3, :], in_=src[:])

    # K=30 bf16 hi/lo decomposition (fp32-accurate):
    #   cross: (-2x)(y) = (mh+ml)(yh+yl), all 4 products
    #   norms: x^2 and y^2 each as 3 bf16 terms
    def split2(flat, scale1, nm):
        base = ft(f"s2b_{nm}", f32)
        nc.vector.tensor_scalar_mul(out=base[:], in0=flat[:], scalar1=scale1)
        h = ft(f"s2h_{nm}", bf16)
        h32 = ft(f"s2h32_{nm}", f32)
        l = ft(f"s2l_{nm}", bf16)
        nc.vector.tensor_scalar_mul(out=h[:], in0=base[:], scalar1=1.0)
        nc.vector.tensor_scalar_mul(out=h32[:], in0=h[:], scalar1=1.0)
        nc.vector.tensor_tensor(out=l[:], in0=base[:], in1=h32[:], op=AL.subtract)
        return h, l

    def split3sq(flat, nm):
        s = ft(f"sq_{nm}", f32)
        nc.vector.tensor_tensor(out=s[:], in0=flat[:], in1=flat[:], op=AL.mult)
        if neg:  # -(x^2): exact bf16 3-way decomposition of the negation
            nc.vector.tensor_scalar_mul(out=s[:], in0=s[:], scalar1=-1.0)
        h = ft(f"s3h_{nm}", bf16)
        h32 = ft(f"s3h32_{nm}", f32)
        d1 = ft(f"s3d1_{nm}", f32)
        m = ft(f"s3m_{nm}", bf16)
        m32 = ft(f"s3m32_{nm}", f32)
        l = ft(f"s3l_{nm}", bf16)
        nc.vector.tensor_scalar_mul(out=h[:], in0=s[:], scalar1=1.0)
        nc.vector.tensor_scalar_mul(out=h32[:], in0=h[:], scalar1=1.0)
        nc.vector.tensor_tensor(out=d1[:], in0=s[:], in1=h32[:], op=AL.subtract)
        nc.vector.tensor_scalar_mul(out=m[:], in0=d1[:], scalar1=1.0)
        nc.vector.tensor_scalar_mul(out=m32[:], in0=m[:], scalar1=1.0)
        nc.vector.tensor_tensor(out=l[:], in0=d1[:], in1=m32[:], op=AL.subtract)
        return h, m, l

    mh, ml = split2(flatX, 2.0 if neg else -2.0, "mx")   # -+2x
    yh, yl = split2(flatY, 1.0, "py")    # y
    sh, sm, sl = split3sq(flatX, "x")    # x^2
    th, tm, tl = split3sq(flatY, "y")    # y^2
    onesf = ft("onesf", bf16)
    nc.vector.tensor_scalar(
        out=onesf[:], in0=flatX[:], scalar1=0.0, scalar2=1.0,
        op0=AL.mult, op1=AL.add,
    )
    o = onesf
    for dst, srcs in (
        (lhsT1, (mh, mh, ml, ml, sh, sm, sl, o, o, o)),
        (rhs1, (yh, yl, yh, yl, o, o, o, th, tm, tl)),
    ):
        for g, src in enumerate(srcs):
            rows(dst, g, src)
    return lhsT1, rhs1


def _build_sp(n=N, ntile=NTILE, gtiles=1, row="tts", evac_bufs=6, reps=1, dbuf=True):
    """Single-pass kernel. gtiles = trailing PSUM tiles per block whose
    column accumulation runs on GpSimd (fp32 SBUF copies; 0 disables
    GpSimd). row = "tts" (fused 2-tile min scan) or "tree" (TT min +
    tensor_reduce fallback)."""
    import concourse.bacc as bacc
    import concourse.mybir as mybir
    from concourse import tile

    f32 = mybir.dt.float32
    bf16 = mybir.dt.bfloat16
    AL = mybir.AluOpType
    AX = mybir.AxisListType

    K = 30
    nblk = n // P            # row blocks (32)
    W = n // ntile           # columns per PSUM tile
    BIG = 1.0e30
    assert ntile % 2 == 0
    npair = ntile // 2
    assert 0 <= gtiles < ntile
    dtiles = ntile - gtiles  # leading tiles: bf16 evac + DVE col accum

    neg = row == "pmr"   # pmr works on u = -t (max-reduce == negated min)
    if neg:
        pmr = _register_pmr()
    mn = AL.max if neg else AL.min

    nc = bacc.Bacc("TRN2", target_bir_lowering=False, debug=False)
    nc._neg = neg
    X_d = nc.dram_tensor("X", [C, n], f32, kind="ExternalInput")
    Y_d = nc.dram_tensor("Y", [C, n], f32, kind="ExternalInput")
    nc._X_d, nc._Y_d = X_d, Y_d
    # single tiny output per core: out[:, 0] = per-lane sum over blocks of
    # the row-stream block minima (neg space when neg), out[:, 1] = per-lane
    # sum of the column minima after the on-device transpose reduce.
    out_d = nc.dram_tensor("out", [P, 2], f32, kind="ExternalOutput")
    assert gtiles == 0, "on-device finish requires gtiles=0"

    with tile.TileContext(nc) as tc:
        with (
            tc.tile_pool(name="big", bufs=1) as big,
            tc.tile_pool(name="small", bufs=1) as small,
            tc.tile_pool(name="evac", bufs=evac_bufs) as evac,
            tc.tile_pool(name="scr", bufs=2) as scr,
            tc.tile_pool(name="psum", bufs=ntile, space="PSUM") as psum,
        ):
            lhsT1, rhs1 = _operands(nc, tc, small, big, n, neg=neg)

            from concourse import masks

            ident = small.tile([P, P], bf16, tag="ident", name="ident")
            masks.make_identity(nc, ident[:])

            # ping-pong accumulator/output sets so one rep's tail (transpose
            # reduce + DMA) overlaps the next rep's compute
            nbuf = 2 if (reps > 1 and dbuf) else 1
            accs = [
                small.tile([P, dtiles * W], bf16, tag=f"acc{j}", name=f"acc{j}")
                for j in range(nbuf)
            ]
            minss = [
                small.tile([P, npair * nblk], f32, tag=f"mins{j}", name=f"mins{j}")
                for j in range(nbuf)
            ]
            minbs = [
                small.tile([P, nblk], f32, tag=f"minb{j}", name=f"minb{j}")
                for j in range(nbuf)
            ]
            d1cols = [
                small.tile([P, nblk], f32, tag=f"d1col{j}", name=f"d1col{j}")
                for j in range(nbuf)
            ]
            outts = [
                small.tile([P, 2], f32, tag=f"outt{j}", name=f"outt{j}")
                for j in range(nbuf)
            ]

            def body(j=0, bi=0):
                acc, accg, mins = accs[j], None, minss[j]
                minb, d1col, outt = minbs[j], d1cols[j], outts[j]
                # no memset: block 0 initializes acc via tensor_copy below
                for i in range(nblk):
                    lw = lhsT1[:, i * P : (i + 1) * P]
                    ets = []

                    def rowop(k):
                        # --- row stream: d2 for this block's 128 rows ---
                        e0, e1 = ets[2 * k], ets[2 * k + 1]
                        mslot = mins[:, npair * i + k : npair * i + k + 1]
                        if row == "pmr":
                            s = scr.tile([P, W], bf16, tag="scr", name=f"scr_{bi}_{i}_{k}")
                            nc.vector._custom_dve(
                                pmr, out=s[:], in0=e0[:], in1=e1[:],
                                accum_out=mslot,
                            )
                        elif row == "tts":
                            s = scr.tile([P, W], f32, tag="scr", name=f"scr_{bi}_{i}_{k}")
                            nc.vector.tensor_tensor_scan(
                                out=s[:], data0=e0[:], data1=e1[:],
                                initial=BIG, op0=AL.min, op1=AL.min,
                            )
                            nc.vector.tensor_scalar_mul(
                                out=mslot, in0=s[:, W - 1 : W], scalar1=1.0
                            )
                        else:  # tree
                            s = scr.tile([P, W], bf16, tag="scr", name=f"scr_{bi}_{i}_{k}")
                            nc.vector.tensor_tensor(
                                out=s[:], in0=e0[:], in1=e1[:], op=AL.min
                            )
                            nc.vector.tensor_reduce(
                                out=mslot, in_=s[:], axis=AX.X, op=AL.min
                            )

                    for t in range(ntile):
                        pt = psum.tile([P, W], f32, tag="pt", name=f"pt_{bi}_{i}_{t}")
                        for c0 in range(0, W, 512):
                            cw = min(512, W - c0)
                            mm_rhs = rhs1[:, t * W + c0 : t * W + c0 + cw]
                            nc.tensor.matmul(
                                pt[:, c0 : c0 + cw], lw, mm_rhs, start=True, stop=True
                            )
                        # ScalarE evacuates PSUM -> SBUF bf16
                        e = evac.tile([P, W], bf16, tag=f"ev{t}", name=f"ev_{bi}_{i}_{t}")
                        nc.scalar.copy(e[:], pt[:])
                        ets.append(e)
                        # --- column stream: running min over row-blocks,
                        # emitted as soon as this tile's copy is queued so
                        # DVE never waits on a whole pair ---
                        av = acc[:, t * W : (t + 1) * W]
                        if i == 0:  # 4x-mode copy doubles as the init
                            nc.vector.tensor_copy(av, e[:])
                        else:
                            nc.vector.tensor_tensor(
                                out=av, in0=av, in1=e[:], op=mn
                            )
                        if t % 2 == 1:
                            rowop(t // 2)

                # --- on-device finish ---
                # d2: per-block min of the pair partials, then per-lane sum
                mv = mins[:].rearrange("p (i k) -> p i k", k=npair)
                nc.vector.tensor_reduce(out=minb[:], in_=mv, axis=AX.X, op=mn)
                nc.vector.reduce_sum(out=outt[:, 0:1], in_=minb[:], axis=AX.X)
                # d1: PE transposes the column accumulator 128 cols at a
                # time into PSUM; DVE reduces the 128 partials per column
                bpc = W // P  # transposed blocks per PSUM chunk
                for c in range(ntile):
                    tp = psum.tile([P, W], bf16, tag="pt", name=f"tp_{bi}_{c}")
                    for b in range(bpc):
                        g = c * bpc + b
                        nc.tensor.transpose(
                            tp[:, b * P : (b + 1) * P],
                            acc[:, g * P : (g + 1) * P],
                            ident[:],
                        )
                    tv = tp[:].rearrange("p (b q) -> p b q", q=P)
                    nc.vector.tensor_reduce(
                        out=d1col[:, c * bpc : (c + 1) * bpc],
                        in_=tv, axis=AX.X, op=mn,
                    )
                nc.vector.reduce_sum(out=outt[:, 1:2], in_=d1col[:], axis=AX.X)
                nc.sync.dma_start(out=out_d[:], in_=outt[:])

            if reps == 1:
                body()
            elif not dbuf:
                with tc.For_i(0, reps, 1):
                    body(0, 0)
            else:
                with tc.For_i(0, reps // 2, 1):
                    body(0, 0)
                    body(1, 1)
                if reps % 2:
                    body(0, 2)

    nc.compile()
    return nc


# ---------------------------------------------------------------------------
# original two-pass kernel (fallback / comparison)

def _build(n=N, mm_dtype="float32r", scan="ttr", evac_bufs=4, reps=1, pe_rot=False, ntile=2):
    import concourse.bacc as bacc
    import concourse.mybir as mybir
    from concourse import tile

    f32 = mybir.dt.float32
    AL = mybir.AluOpType
    AX = mybir.AxisListType
    opdt = mybir.dt.float32r if mm_dtype == "float32r" else f32

    bf16 = mybir.dt.bfloat16
    bf16x = mm_dtype == "bf16x"
    K = 30 if bf16x else 9   # contraction rows
    nblk = n // P            # row blocks per pass
    W = n // ntile           # columns per PSUM tile
    PF = (C * n) // P        # flat layout partition count (96 for n=4096)
    nacc = 2 if scan == "ttr" else ntile
    BIG = 1.0e30

    nc = bacc.Bacc("TRN2", target_bir_lowering=False, debug=False)
    X_d = nc.dram_tensor("X", [C, n], f32, kind="ExternalInput")
    Y_d = nc.dram_tensor("Y", [C, n], f32, kind="ExternalInput")
    out_d = nc.dram_tensor("out", [P, 2], f32, kind="ExternalOutput")

    with tile.TileContext(nc) as tc:
        with (
            tc.tile_pool(name="big", bufs=1) as big,
            tc.tile_pool(name="small", bufs=1) as small,
            tc.tile_pool(name="evac", bufs=evac_bufs) as evac,
            tc.tile_pool(name="psum", bufs=ntile, space="PSUM") as psum,
        ):
            kdt = bf16 if bf16x else opdt
            lhsT1 = big.tile([K, n], kdt, tag="lhsT1")
            rhs1 = big.tile([K, n], kdt, tag="rhs1")
            lhsT2 = big.tile([K, n], kdt, tag="lhsT2")
            rhs2 = big.tile([K, n], kdt, tag="rhs2")

            flatX = small.tile([PF, P], f32, tag="flatX")
            flatY = small.tile([PF, P], f32, tag="flatY")

            mins1 = small.tile([P, nacc * nblk], f32, tag="mins1")
            mins2 = small.tile([P, nacc * nblk], f32, tag="mins2")
            minb1 = small.tile([P, nblk], f32, tag="minb1")
            minb2 = small.tile([P, nblk], f32, tag="minb2")
            outt = small.tile([P, 2], f32, tag="outt")

            xf_src = X_d[:].rearrange("c n -> (c n)").rearrange("(p f) -> p f", f=P)
            yf_src = Y_d[:].rearrange("c n -> (c n)").rearrange("(p f) -> p f", f=P)
            nc.sync.dma_start(out=flatX[:], in_=xf_src)
            nc.sync.dma_start(out=flatY[:], in_=yf_src)

            def ft(name, dtype):
                return small.tile([PF, P], dtype, tag=name, name=name)

            def rows(dst, g, src):
                nc.sync.dma_start(out=dst[3 * g : 3 * g + 3, :], in_=src[:])

            def split2(flat, scale1, nm):
                base = ft(f"s2b_{nm}", f32)
                nc.vector.tensor_scalar_mul(out=base[:], in0=flat[:], scalar1=scale1)
                h = ft(f"s2h_{nm}", bf16)
                h32 = ft(f"s2h32_{nm}", f32)
                l = ft(f"s2l_{nm}", bf16)
                nc.vector.tensor_scalar_mul(out=h[:], in0=base[:], scalar1=1.0)
                nc.vector.tensor_scalar_mul(out=h32[:], in0=h[:], scalar1=1.0)
                nc.vector.tensor_tensor(out=l[:], in0=base[:], in1=h32[:], op=AL.subtract)
                return h, l

            def split3sq(flat, nm):
                s = ft(f"sq_{nm}", f32)
                nc.vector.tensor_tensor(out=s[:], in0=flat[:], in1=flat[:], op=AL.mult)
                h = ft(f"s3h_{nm}", bf16)
                h32 = ft(f"s3h32_{nm}", f32)
                d1 = ft(f"s3d1_{nm}", f32)
                m = ft(f"s3m_{nm}", bf16)
                m32 = ft(f"s3m32_{nm}", f32)
                l = ft(f"s3l_{nm}", bf16)
                nc.vector.tensor_scalar_mul(out=h[:], in0=s[:], scalar1=1.0)
                nc.vector.tensor_scalar_mul(out=h32[:], in0=h[:], scalar1=1.0)
                nc.vector.tensor_tensor(out=d1[:], in0=s[:], in1=h32[:], op=AL.subtract)
                nc.vector.tensor_scalar_mul(out=m[:], in0=d1[:], scalar1=1.0)
                nc.vector.tensor_scalar_mul(out=m32[:], in0=m[:], scalar1=1.0)
                nc.vector.tensor_tensor(out=l[:], in0=d1[:], in1=m32[:], op=AL.subtract)
                return h, m, l

            mh, ml = split2(flatX, -2.0, "mx")   # -2x
            nh, nl = split2(flatY, -2.0, "my")   # -2y
            xh, xl = split2(flatX, 1.0, "px")    # x
            yh, yl = split2(flatY, 1.0, "py")    # y
            sh, sm, sl = split3sq(flatX, "x")  # x^2
            th, tm, tl = split3sq(flatY, "y")  # y^2
            onesf = ft("onesf", bf16)
            nc.vector.tensor_scalar(
                out=onesf[:], in0=flatX[:], scalar1=0.0, scalar2=1.0,
                op0=AL.mult, op1=AL.add,
            )
            o = onesf
            for dst, srcs in (
                (lhsT1, (mh, mh, ml, ml, sh, sm, sl, o, o, o)),
                (rhs1, (yh, yl, yh, yl, o, o, o, th, tm, tl)),
                (lhsT2, (nh, nh, nl, nl, th, tm, tl, o, o, o)),
                (rhs2, (xh, xl, xh, xl, o, o, o, sh, sm, sl)),
            ):
                for g, src in enumerate(srcs):
                    rows(dst, g, src)

            def do_pass(lhsT, rhs, mins):
                for i in range(nblk):
                    lw = lhsT[:, i * P : (i + 1) * P]
                    pts = []
                    for t in range(ntile):
                        pt = psum.tile([P, W], f32, tag="pt", name=f"pt_{i}_{t}")
                        for c0 in range(0, W, 512):
                            cw = min(512, W - c0)
                            mm_rhs = rhs[:, t * W + c0 : t * W + c0 + cw]
                            nc.tensor.matmul(
                                pt[:, c0 : c0 + cw], lw, mm_rhs, start=True, stop=True
                            )
                        pts.append(pt)
                    for t in range(ntile):
                        nc.vector.tensor_reduce(
                            out=mins[:, ntile * i + t : ntile * i + t + 1],
                            in_=pts[t][:],
                            axis=AX.X,
                            op=AL.min,
                        )

            def body():
                do_pass(lhsT1, rhs1, mins1)
                do_pass(lhsT2, rhs2, mins2)
                for pi, (mins, minb) in enumerate(((mins1, minb1), (mins2, minb2))):
                    mv = mins[:].rearrange("p (i k) -> p i k", k=nacc)
                    nc.vector.tensor_reduce(out=minb[:], in_=mv, axis=AX.X, op=AL.min)
                    nc.vector.reduce_sum(
                        out=outt[:, pi : pi + 1], in_=minb[:], axis=AX.X
                    )

            if reps == 1:
                body()
            else:
                with tc.For_i(0, reps, 1):
                    body()

            nc.sync.dma_start(out=out_d[:], in_=outt[:])

    nc.compile()
    return nc


# Best hardware-validated configuration.
BEST = dict(mode="sp", ntile=NTILE, gtiles=0, row="pmr", evac_bufs=8, dbuf=False)


def _program(**kw):
    cfg = dict(BEST)
    cfg.update(kw)
    key = tuple(sorted(cfg.items()))
    if key not in _cache:
        mode = cfg.pop("mode", "sp")
        if mode == "sp":
            _cache[key] = _build_sp(**cfg)
        else:
            # old two-pass builder takes a different knob set
            old = {k: v for k, v in cfg.items()
                   if k in ("n", "mm_dtype", "scan", "evac_bufs", "reps", "ntile")}
            _cache[key] = _build(**old)
        _cache[key]._mode = mode
    return _cache[key]


def kernel(X, Y, ps=None, **kw):
    from concourse.bass_utils import run_bass_kernel_spmd

    X = np.asarray(X, dtype=np.float32)
    Y = np.asarray(Y, dtype=np.float32)
    assert X.shape == (B, C, N) and Y.shape == (B, C, N)

    nc = _program(**kw)
    in_maps = [
        {"X": np.ascontiguousarray(X[b]), "Y": np.ascontiguousarray(Y[b])}
        for b in range(B)
    ]
    res = run_bass_kernel_spmd(nc, in_maps, list(range(B)))
    total = 0.0
    if getattr(nc, "_mode", "sp") == "sp":
        # out holds per-lane sums of block/column minima, negated when the
        # kernel ran on the negated matrix; sgn converts back to d-space.
        sgn = -1.0 if getattr(nc, "_neg", False) else 1.0
        for r in res.results:
            total += sgn * np.asarray(r["out"]).astype(np.float64).sum()
    else:
        for r in res.results:
            total += r["out"].astype(np.float64).sum()
    return np.float32(total / (2.0 * B * N))
